# revision 8
# baseline (speedup 1.0000x reference)
"""Trainium2 Bass kernel for the input-attention LSTM encoder (DA-RNN style).

Shapes (hardcoded): B=512, T=128, N=256, M=128. 8 NeuronCores, data-parallel
over batch (B_loc=64 per core), recurrent T-loop local per core.

Per core layout:
  P_sb  [s=128, b=64, n=256]  feat_proj, s on partitions (SBUF resident)
  XT_sb [n'=128, h=2, b=64, t=128]  X transposed (for x_t in [n, b] layout)
  state h_T, c_T [feat=128, b=64]  (feature-major so gate bias is per-partition)

Per step t:
  a   = W_hs_h @ h + W_hs_c @ c                  (PE)    [s, b]
  Z   = P + bcast(a)                             (DVE)   [s, b, n]
  Y   = tanh(Z) -> bf16                          (ACT)
  E_T[p, 2b+h_half] = sum_s v_s Y[s, b, 128h+p]  (PE, 128 small matmuls,
                                                  v split hi/lo bf16, N=2)
  expE = exp(E_hi) * exp(E_lo)                   (ACT + DVE)
  softmax denom via ones/pairmat matmuls + DVE reciprocal
  x_tilde = X[:, :, t] * expE * (1/S)            (DVE)
  gates = W_ih @ x_tilde + W_hh @ h  (+bias via ACT per-partition bias)
  sigmoid via 0.5 + 0.5*tanh(x/2)  -> LSTM update (DVE/ACT)
"""

import os
import numpy as np
import ml_dtypes

import concourse.bacc as bacc
import concourse.bass as bass
import concourse.mybir as mybir
import concourse.tile as tile
from concourse.bass_utils import run_bass_kernel_spmd

f32 = mybir.dt.float32
bf16 = mybir.dt.bfloat16
AF = mybir.ActivationFunctionType
ALU = mybir.AluOpType

B, T, N, M = 512, 128, 256, 128
NCORES = 8
BL = B // NCORES          # 64 batch per core
NCH = 4                   # chunks per step over b (16 b's each)
BCH = BL // NCH           # 16
T_STEPS = int(os.environ.get("K_STEPS", str(T)))


def _build(trace_friendly=False):
    nc = bacc.Bacc("TRN2", target_bir_lowering=False)

    X_in = nc.dram_tensor("x", [BL, T, N], f32, kind="ExternalInput")
    W_xt = nc.dram_tensor("w_xt", [128, 128], f32, kind="ExternalInput")
    W_hst = nc.dram_tensor("w_hst", [128, 2, 128], f32, kind="ExternalInput")
    W_iht = nc.dram_tensor("w_iht", [128, 2, 4, 128], f32, kind="ExternalInput")
    W_hht = nc.dram_tensor("w_hht", [128, 4, 128], f32, kind="ExternalInput")
    V_pair = nc.dram_tensor("v_pair", [128, 2], bf16, kind="ExternalInput")
    HBias = nc.dram_tensor("hbias", [128, 4], f32, kind="ExternalInput")
    FBias = nc.dram_tensor("fbias", [128, 4], f32, kind="ExternalInput")
    Ident = nc.dram_tensor("ident", [128, 128], f32, kind="ExternalInput")
    OnesC = nc.dram_tensor("ones_col", [128, 1], f32, kind="ExternalInput")
    OnesR = nc.dram_tensor("ones_row", [1, 128], f32, kind="ExternalInput")
    PairM = nc.dram_tensor("pairmat", [128, BL], f32, kind="ExternalInput")
    H_out = nc.dram_tensor("h_out", [BL, T, M], f32, kind="ExternalOutput")

    with tile.TileContext(nc) as tc:
        with tc.tile_pool(name="const", bufs=1) as cpool, \
             tc.tile_pool(name="big", bufs=1) as bigpool, \
             tc.tile_pool(name="work", bufs=3) as work, \
             tc.tile_pool(name="ybuf", bufs=3) as ybuf, \
             tc.tile_pool(name="small", bufs=3) as small, \
             tc.tile_pool(name="state", bufs=3) as statep, \
             tc.tile_pool(name="stage", bufs=2) as stagep, \
             tc.tile_pool(name="ps_e", bufs=2, space="PSUM") as psp_e, \
             tc.tile_pool(name="ps_g", bufs=2, space="PSUM") as psp_g, \
             tc.tile_pool(name="ps_a", bufs=2, space="PSUM") as psp_a, \
             tc.tile_pool(name="ps_m", bufs=2, space="PSUM") as psp_m:

            # ---- constants to SBUF ----
            w_xt = cpool.tile([128, 128], f32)
            w_hst = cpool.tile([128, 2, 128], f32)
            w_iht = cpool.tile([128, 2, 4, 128], f32)
            w_hht = cpool.tile([128, 4, 128], f32)
            v_pair = cpool.tile([128, 2], bf16)
            hbias = cpool.tile([128, 4], f32)
            fbias = cpool.tile([128, 4], f32)
            ident = cpool.tile([128, 128], f32)
            ones_col = cpool.tile([128, 1], f32)
            ones_row = cpool.tile([1, 128], f32)
            pairmat = cpool.tile([128, BL], f32)
            for dst, src in [(w_xt, W_xt), (w_hst, W_hst), (w_iht, W_iht),
                             (w_hht, W_hht), (v_pair, V_pair), (hbias, HBias),
                             (fbias, FBias), (ident, Ident), (ones_col, OnesC),
                             (ones_row, OnesR), (pairmat, PairM)]:
                nc.sync.dma_start(dst[:], src[:])

            P_sb = bigpool.tile([128, BL, N], bf16)      # [s, b, n] bf16
            XT_sb = bigpool.tile([128, 2, BL, T], f32)   # [n', h, b, t]

            # ---- preamble: load X, compute P = W_x @ X_b, transpose X ----
            for q in range(NCH):
                b0 = q * BCH
                xtn = work.tile([128, BCH, N], f32, tag="work")
                nc.sync.dma_start(
                    xtn[:], X_in[b0:b0 + BCH].rearrange("b t n -> t b n"))
                # P for 2 b's at a time (N=512 moving limit)
                for i in range(BCH // 2):
                    pp = psp_e.tile([128, 512], f32, tag="e")
                    nc.tensor.matmul(
                        pp[:], w_xt[:],
                        xtn[:, 2 * i:2 * i + 2, :].rearrange("p b n -> p (b n)"),
                        start=True, stop=True)
                    nc.scalar.copy(
                        P_sb[:, b0 + 2 * i:b0 + 2 * i + 2, :]
                        .rearrange("p b n -> p (b n)"), pp[:])
                # transpose X[b] -> XT, batches of 4 [128,128] blocks per copy
                for i in range(BCH // 2):
                    tp = psp_g.tile([128, 4, 128], f32, tag="g")
                    for j in range(2):      # b-offset within pair
                        for h in range(2):  # n half
                            nc.tensor.transpose(
                                tp[:, 2 * j + h, :],
                                xtn[:, 2 * i + j, 128 * h:128 * h + 128],
                                ident[:])
                    bb = b0 + 2 * i
                    nc.vector.tensor_copy(
                        XT_sb[:, :, bb:bb + 2, :].rearrange("p h b t -> p b h t"),
                        tp[:].rearrange("p (b h) t -> p b h t", b=2))

            # ---- state init ----
            h_T = statep.tile([128, BL], f32, tag="hT")
            c_T = statep.tile([128, BL], f32, tag="cT")
            nc.vector.memset(h_T[:], 0.0)
            nc.vector.memset(c_T[:], 0.0)

            # ---- recurrent steps ----
            for t_raw in range(T_STEPS):
                t = t_raw % T
                # a[s, b] = W_hs_h @ h + W_hs_c @ c
                a_ps = psp_a.tile([128, BL], f32, tag="a")
                nc.tensor.matmul(a_ps[:], w_hst[:, 0, :], h_T[:],
                                 start=True, stop=False)
                nc.tensor.matmul(a_ps[:], w_hst[:, 1, :], c_T[:],
                                 start=False, stop=True)
                a2 = small.tile([128, BL, 2], bf16, tag="a2")
                nc.scalar.copy(a2[:], a_ps[:].broadcast_to((128, BL, 2)))

                e_ps = psp_e.tile([128, 128, 2], f32, tag="e")
                for k in range(NCH):
                    bk = k * BCH
                    z = work.tile([128, BCH, N], bf16, tag="work")
                    a_bc = (a2[:, bk:bk + BCH, :]
                            .broadcast_to((128, BCH, 2, N // 2))
                            .rearrange("p b two r -> p b r two"))
                    nc.vector.tensor_tensor(
                        out=z[:].rearrange("p b (r two) -> p b r two", two=2),
                        in0=P_sb[:, bk:bk + BCH, :]
                        .rearrange("p b (r two) -> p b r two", two=2),
                        in1=a_bc, op=ALU.add)
                    y = ybuf.tile([128, BCH * N], bf16, tag="y")
                    nc.scalar.activation(y[:], z[:].rearrange("p b n -> p (b n)"),
                                         AF.Tanh)
                    for c in range(BCH * N // 128):  # 32 col-blocks of 128
                        cc = k * 32 + c
                        nc.tensor.matmul(e_ps[:, cc, :],
                                         y[:, 128 * c:128 * c + 128],
                                         v_pair[:], start=True, stop=True)

                # softmax pieces
                expp = small.tile([128, 128, 2], f32, tag="expp")
                nc.scalar.activation(expp[:], e_ps[:], AF.Exp)
                expE = small.tile([128, 128], f32, tag="expE")
                nc.vector.tensor_tensor(out=expE[:], in0=expp[:, :, 0],
                                        in1=expp[:, :, 1], op=ALU.mult)
                misc = psp_m.tile([128, 512], f32, tag="m")
                s2_ps = misc[:, 0:1]
                nc.tensor.matmul(s2_ps, expE[:], ones_col[:],
                                 start=True, stop=True)
                s2_sb = small.tile([128, 1], f32, tag="s2sb")
                nc.vector.tensor_copy(s2_sb[:], s2_ps)
                s_ps = misc[0:1, 64:64 + BL]
                nc.tensor.matmul(s_ps, s2_sb[:], pairmat[:],
                                 start=True, stop=True)
                r_sb = small.tile([1, BL], f32, tag="r")
                nc.vector.reciprocal(r_sb[:], s_ps)
                rrep_ps = misc[:, 128:128 + BL]
                nc.tensor.matmul(rrep_ps, ones_row[:], r_sb[:],
                                 start=True, stop=True)

                # x_tilde[h][n', b] = X[n, b, t] * expE[n', 2b+h] / S[b]
                u_sb = small.tile([128, 2, BL], f32, tag="u")
                nc.vector.tensor_tensor(
                    out=u_sb[:], in0=XT_sb[:, :, :, t],
                    in1=expE[:].rearrange("p (b h) -> p h b", h=2),
                    op=ALU.mult)
                xt_sb = small.tile([128, 2, BL], f32, tag="xt")
                nc.vector.tensor_tensor(
                    out=xt_sb[:], in0=u_sb[:],
                    in1=rrep_ps.broadcast_to((128, BL, 2))
                    .rearrange("p b h -> p h b"),
                    op=ALU.mult)

                # gates[j, b] = W_ih @ x_tilde + W_hh @ h
                g_ps = psp_g.tile([128, 4, BL], f32, tag="g")
                for q in range(4):
                    nc.tensor.matmul(g_ps[:, q, :], w_hht[:, q, :], h_T[:],
                                     start=True, stop=False)
                    nc.tensor.matmul(g_ps[:, q, :], w_iht[:, 0, q, :],
                                     xt_sb[:, 0, :], start=False, stop=False)
                    nc.tensor.matmul(g_ps[:, q, :], w_iht[:, 1, q, :],
                                     xt_sb[:, 1, :], start=False, stop=True)

                # gate activations: sigmoid(x) = 0.5 + 0.5 tanh(x/2)
                tg = small.tile([128, 4, BL], f32, tag="tg")
                for q in (0, 1, 3):
                    nc.scalar.activation(tg[:, q, :], g_ps[:, q, :], AF.Tanh,
                                         bias=hbias[:, q:q + 1], scale=0.5)
                nc.scalar.activation(tg[:, 2, :], g_ps[:, 2, :], AF.Tanh,
                                     bias=fbias[:, 2:3], scale=1.0)
                ug = small.tile([128, 3, BL], f32, tag="ug")  # u_i, u_f, u_o
                for qi, q in enumerate((0, 1, 3)):
                    nc.vector.tensor_scalar(out=ug[:, qi, :], in0=tg[:, q, :],
                                            scalar1=0.5, scalar2=0.5,
                                            op0=ALU.mult, op1=ALU.add)

                m1 = small.tile([128, BL], f32, tag="m1")
                nc.vector.tensor_tensor(out=m1[:], in0=ug[:, 1, :], in1=c_T[:],
                                        op=ALU.mult)
                m2 = small.tile([128, BL], f32, tag="m2")
                nc.vector.tensor_tensor(out=m2[:], in0=ug[:, 0, :],
                                        in1=tg[:, 2, :], op=ALU.mult)
                c_new = statep.tile([128, BL], f32, tag="cT")
                nc.vector.tensor_tensor(out=c_new[:], in0=m1[:], in1=m2[:],
                                        op=ALU.add)
                tc2 = small.tile([128, BL], f32, tag="tc2")
                nc.scalar.activation(tc2[:], c_new[:], AF.Tanh)
                h_new = statep.tile([128, BL], f32, tag="hT")
                nc.vector.tensor_tensor(out=h_new[:], in0=ug[:, 2, :],
                                        in1=tc2[:], op=ALU.mult)
                h_T, c_T = h_new, c_new

                # output staging: h2_bt = h_T.T -> stage, DMA every 8 steps
                hbt_ps = misc[0:BL, 192:320]
                nc.tensor.transpose(hbt_ps, h_T[:], ident[:])
                if t % 8 == 0:
                    stage = stagep.tile([BL, 8, 128], f32, tag="stage")
                nc.vector.tensor_copy(stage[:, t % 8, :], hbt_ps)
                if t % 8 == 7 or t == T_STEPS - 1:
                    t0 = (t // 8) * 8
                    nc.sync.dma_start(H_out[:, t0:t + 1, :],
                                      stage[:, :t + 1 - t0, :])

    nc.finalize()
    return nc


_NC_CACHE = {}


def _get_nc():
    if "nc" not in _NC_CACHE:
        _NC_CACHE["nc"] = _build()
    return _NC_CACHE["nc"]


def _prep_weights(W_e, v_e, W_ih, W_hh, b_ih, b_hh):
    W_hs, W_x = W_e[:, :2 * M], W_e[:, 2 * M:]
    W_hsT = np.ascontiguousarray(W_hs.T)             # [2m, s]
    w_hst = np.ascontiguousarray(
        W_hsT.reshape(2, 128, 128).transpose(1, 0, 2))  # [j, c, s]
    w_xt = np.ascontiguousarray(W_x.T)               # [t, s]
    W_ihT = np.ascontiguousarray(W_ih.T)             # [n=256, j=512]
    w_iht = np.ascontiguousarray(
        W_ihT.reshape(2, 128, 4, 128).transpose(1, 0, 2, 3))  # [n', h, q, j']
    W_hhT = np.ascontiguousarray(W_hh.T)             # [m, j=512]
    w_hht = np.ascontiguousarray(
        W_hhT.reshape(128, 4, 128))                  # [m, q, j']
    v = v_e[0].astype(np.float32)
    v_hi = v.astype(ml_dtypes.bfloat16)
    v_lo = (v - v_hi.astype(np.float32)).astype(ml_dtypes.bfloat16)
    v_pair = np.ascontiguousarray(np.stack([v_hi, v_lo], axis=1))
    bias = (b_ih + b_hh).astype(np.float32)
    hbias = np.ascontiguousarray(0.5 * bias.reshape(4, 128).T)
    fbias = np.ascontiguousarray(bias.reshape(4, 128).T)
    ident = np.eye(128, dtype=np.float32)
    ones_col = np.ones((128, 1), np.float32)
    ones_row = np.ones((1, 128), np.float32)
    pairmat = np.zeros((128, BL), np.float32)
    pairmat[np.arange(128), np.arange(128) // 2] = 1.0
    return dict(w_xt=w_xt, w_hst=w_hst, w_iht=w_iht, w_hht=w_hht,
                v_pair=v_pair, hbias=hbias, fbias=fbias, ident=ident,
                ones_col=ones_col, ones_row=ones_row, pairmat=pairmat)


def kernel(X, W_e, v_e, W_ih, W_hh, b_ih, b_hh, _trace=False, _tmpdir=None):
    X = np.ascontiguousarray(np.asarray(X, dtype=np.float32))
    wd = _prep_weights(np.asarray(W_e, np.float32), np.asarray(v_e, np.float32),
                       np.asarray(W_ih, np.float32), np.asarray(W_hh, np.float32),
                       np.asarray(b_ih, np.float32), np.asarray(b_hh, np.float32))
    nc = _get_nc()
    in_maps = []
    for core in range(NCORES):
        m = dict(wd)
        m["x"] = np.ascontiguousarray(X[core * BL:(core + 1) * BL])
        in_maps.append(m)
    kw = {}
    if _trace:
        kw = dict(trace=True, tmpdir=_tmpdir)
    res = run_bass_kernel_spmd(nc, in_maps, core_ids=list(range(NCORES)), **kw)
    out = np.concatenate(
        [res.results[c]["h_out"].transpose(1, 0, 2) for c in range(NCORES)],
        axis=1)
    if _trace:
        return out, res
    return out


# revision 9
# speedup vs baseline: 1.0747x; 1.0747x over previous
"""Trainium2 Bass kernel for the input-attention LSTM encoder (DA-RNN style).

Shapes (hardcoded): B=512, T=128, N=256, M=128. 8 NeuronCores, data-parallel
over batch (B_loc=64 per core), recurrent T-loop local per core.

Per core layout:
  P_sb  [s=128, b=64, n=256]  feat_proj, s on partitions (SBUF resident)
  XT_sb [n'=128, h=2, b=64, t=128]  X transposed (for x_t in [n, b] layout)
  state h_T, c_T [feat=128, b=64]  (feature-major so gate bias is per-partition)

Per step t:
  a   = W_hs_h @ h + W_hs_c @ c                  (PE)    [s, b]
  Z   = P + bcast(a)                             (DVE)   [s, b, n]
  Y   = tanh(Z) -> bf16                          (ACT)
  E_T[p, 2b+h_half] = sum_s v_s Y[s, b, 128h+p]  (PE, 128 small matmuls,
                                                  v split hi/lo bf16, N=2)
  expE = exp(E_hi) * exp(E_lo)                   (ACT + DVE)
  softmax denom via ones/pairmat matmuls + DVE reciprocal
  x_tilde = X[:, :, t] * expE * (1/S)            (DVE)
  gates = W_ih @ x_tilde + W_hh @ h  (+bias via ACT per-partition bias)
  sigmoid via 0.5 + 0.5*tanh(x/2)  -> LSTM update (DVE/ACT)
"""

import os
import numpy as np
import ml_dtypes

import concourse.bacc as bacc
import concourse.bass as bass
import concourse.mybir as mybir
import concourse.tile as tile
from concourse.bass_utils import run_bass_kernel_spmd

f32 = mybir.dt.float32
bf16 = mybir.dt.bfloat16
AF = mybir.ActivationFunctionType
ALU = mybir.AluOpType

B, T, N, M = 512, 128, 256, 128
NCORES = 8
BL = B // NCORES          # 64 batch per core
NCH = 8                   # chunks per step over b (8 b's each)
BCH = BL // NCH           # 16
T_STEPS = int(os.environ.get("K_STEPS", str(T)))


def _build(trace_friendly=False):
    nc = bacc.Bacc("TRN2", target_bir_lowering=False)

    X_in = nc.dram_tensor("x", [BL, T, N], f32, kind="ExternalInput")
    W_xt = nc.dram_tensor("w_xt", [128, 128], f32, kind="ExternalInput")
    W_hst = nc.dram_tensor("w_hst", [128, 2, 128], f32, kind="ExternalInput")
    W_iht = nc.dram_tensor("w_iht", [128, 2, 4, 128], f32, kind="ExternalInput")
    W_hht = nc.dram_tensor("w_hht", [128, 4, 128], f32, kind="ExternalInput")
    V_pair = nc.dram_tensor("v_pair", [128, 2], bf16, kind="ExternalInput")
    HBias = nc.dram_tensor("hbias", [128, 4], f32, kind="ExternalInput")
    FBias = nc.dram_tensor("fbias", [128, 4], f32, kind="ExternalInput")
    Ident = nc.dram_tensor("ident", [128, 128], f32, kind="ExternalInput")
    OnesC = nc.dram_tensor("ones_col", [128, 1], f32, kind="ExternalInput")
    OnesR = nc.dram_tensor("ones_row", [1, 128], f32, kind="ExternalInput")
    PairM = nc.dram_tensor("pairmat", [128, BL], f32, kind="ExternalInput")
    H_out = nc.dram_tensor("h_out", [BL, T, M], f32, kind="ExternalOutput")

    with tile.TileContext(nc) as tc:
        with tc.tile_pool(name="const", bufs=1) as cpool, \
             tc.tile_pool(name="big", bufs=1) as bigpool, \
             tc.tile_pool(name="work", bufs=3) as work, \
             tc.tile_pool(name="ybuf", bufs=3) as ybuf, \
             tc.tile_pool(name="small", bufs=3) as small, \
             tc.tile_pool(name="state", bufs=3) as statep, \
             tc.tile_pool(name="stage", bufs=2) as stagep, \
             tc.tile_pool(name="ps_e", bufs=2, space="PSUM") as psp_e, \
             tc.tile_pool(name="ps_g", bufs=2, space="PSUM") as psp_g, \
             tc.tile_pool(name="ps_a", bufs=2, space="PSUM") as psp_a, \
             tc.tile_pool(name="ps_m", bufs=2, space="PSUM") as psp_m:

            # ---- constants to SBUF ----
            w_xt = cpool.tile([128, 128], f32)
            w_hst = cpool.tile([128, 2, 128], f32)
            w_iht = cpool.tile([128, 2, 4, 128], f32)
            w_hht = cpool.tile([128, 4, 128], f32)
            v_pair = cpool.tile([128, 2], bf16)
            hbias = cpool.tile([128, 4], f32)
            fbias = cpool.tile([128, 4], f32)
            ident = cpool.tile([128, 128], f32)
            ones_col = cpool.tile([128, 1], f32)
            ones_row = cpool.tile([1, 128], f32)
            pairmat = cpool.tile([128, BL], f32)
            for dst, src in [(w_xt, W_xt), (w_hst, W_hst), (w_iht, W_iht),
                             (w_hht, W_hht), (v_pair, V_pair), (hbias, HBias),
                             (fbias, FBias), (ident, Ident), (ones_col, OnesC),
                             (ones_row, OnesR), (pairmat, PairM)]:
                nc.sync.dma_start(dst[:], src[:])

            P_sb = bigpool.tile([128, BL, N], bf16)      # [s, b, n] bf16
            XT_sb = bigpool.tile([128, 2, BL, T], f32)   # [n', h, b, t]

            # ---- preamble: load X, compute P = W_x @ X_b, transpose X ----
            for q in range(NCH):
                b0 = q * BCH
                xtn = work.tile([128, BCH, N], f32, tag="work")
                nc.sync.dma_start(
                    xtn[:], X_in[b0:b0 + BCH].rearrange("b t n -> t b n"))
                # P for 2 b's at a time (N=512 moving limit)
                for i in range(BCH // 2):
                    pp = psp_e.tile([128, 512], f32, tag="e")
                    nc.tensor.matmul(
                        pp[:], w_xt[:],
                        xtn[:, 2 * i:2 * i + 2, :].rearrange("p b n -> p (b n)"),
                        start=True, stop=True)
                    nc.scalar.copy(
                        P_sb[:, b0 + 2 * i:b0 + 2 * i + 2, :]
                        .rearrange("p b n -> p (b n)"), pp[:])
                # transpose X[b] -> XT, batches of 4 [128,128] blocks per copy
                for i in range(BCH // 2):
                    tp = psp_g.tile([128, 4, 128], f32, tag="g")
                    for j in range(2):      # b-offset within pair
                        for h in range(2):  # n half
                            nc.tensor.transpose(
                                tp[:, 2 * j + h, :],
                                xtn[:, 2 * i + j, 128 * h:128 * h + 128],
                                ident[:])
                    bb = b0 + 2 * i
                    nc.vector.tensor_copy(
                        XT_sb[:, :, bb:bb + 2, :].rearrange("p h b t -> p b h t"),
                        tp[:].rearrange("p (b h) t -> p b h t", b=2))

            # ---- state init ----
            h_T = statep.tile([128, BL], f32, tag="hT")
            c_T = statep.tile([128, BL], f32, tag="cT")
            nc.vector.memset(h_T[:], 0.0)
            nc.vector.memset(c_T[:], 0.0)

            # ---- recurrent steps ----
            for t_raw in range(T_STEPS):
                t = t_raw % T
                # a[s, b] = W_hs_h @ h + W_hs_c @ c
                a_ps = psp_a.tile([128, BL], f32, tag="a")
                nc.tensor.matmul(a_ps[:], w_hst[:, 0, :], h_T[:],
                                 start=True, stop=False)
                nc.tensor.matmul(a_ps[:], w_hst[:, 1, :], c_T[:],
                                 start=False, stop=True)
                a2 = small.tile([128, BL, 2], bf16, tag="a2")
                nc.scalar.copy(a2[:], a_ps[:].broadcast_to((128, BL, 2)))

                e_ps = psp_e.tile([128, 128, 2], f32, tag="e")
                for k in range(NCH):
                    bk = k * BCH
                    z = work.tile([128, BCH, N], bf16, tag="work")
                    a_bc = (a2[:, bk:bk + BCH, :]
                            .broadcast_to((128, BCH, 2, N // 2))
                            .rearrange("p b two r -> p b r two"))
                    nc.vector.tensor_tensor(
                        out=z[:].rearrange("p b (r two) -> p b r two", two=2),
                        in0=P_sb[:, bk:bk + BCH, :]
                        .rearrange("p b (r two) -> p b r two", two=2),
                        in1=a_bc, op=ALU.add)
                    y = ybuf.tile([128, BCH * N], bf16, tag="y")
                    nc.scalar.activation(y[:], z[:].rearrange("p b n -> p (b n)"),
                                         AF.Tanh)
                    for c in range(BCH * N // 128):  # col-blocks of 128
                        cc = k * (BCH * N // 128) + c
                        nc.tensor.matmul(e_ps[:, cc, :],
                                         y[:, 128 * c:128 * c + 128],
                                         v_pair[:], start=True, stop=True)

                # softmax pieces
                expp = small.tile([128, 128, 2], f32, tag="expp")
                nc.scalar.activation(expp[:], e_ps[:], AF.Exp)
                expE = small.tile([128, 128], f32, tag="expE")
                nc.vector.tensor_tensor(out=expE[:], in0=expp[:, :, 0],
                                        in1=expp[:, :, 1], op=ALU.mult)
                misc = psp_m.tile([128, 512], f32, tag="m")
                s2_ps = misc[:, 0:1]
                nc.tensor.matmul(s2_ps, expE[:], ones_col[:],
                                 start=True, stop=True)
                s2_sb = small.tile([128, 1], f32, tag="s2sb")
                nc.vector.tensor_copy(s2_sb[:], s2_ps)
                s_ps = misc[0:1, 64:64 + BL]
                nc.tensor.matmul(s_ps, s2_sb[:], pairmat[:],
                                 start=True, stop=True)
                r_sb = small.tile([1, BL], f32, tag="r")
                nc.vector.reciprocal(r_sb[:], s_ps)
                rrep_ps = misc[:, 128:128 + BL]
                nc.tensor.matmul(rrep_ps, ones_row[:], r_sb[:],
                                 start=True, stop=True)

                # x_tilde[h][n', b] = X[n, b, t] * expE[n', 2b+h] / S[b]
                u_sb = small.tile([128, 2, BL], f32, tag="u")
                nc.vector.tensor_tensor(
                    out=u_sb[:], in0=XT_sb[:, :, :, t],
                    in1=expE[:].rearrange("p (b h) -> p h b", h=2),
                    op=ALU.mult)
                xt_sb = small.tile([128, 2, BL], f32, tag="xt")
                nc.vector.tensor_tensor(
                    out=xt_sb[:], in0=u_sb[:],
                    in1=rrep_ps.broadcast_to((128, BL, 2))
                    .rearrange("p b h -> p h b"),
                    op=ALU.mult)

                # gates[j, b] = W_ih @ x_tilde + W_hh @ h
                g_ps = psp_g.tile([128, 4, BL], f32, tag="g")
                for q in range(4):
                    nc.tensor.matmul(g_ps[:, q, :], w_hht[:, q, :], h_T[:],
                                     start=True, stop=False)
                    nc.tensor.matmul(g_ps[:, q, :], w_iht[:, 0, q, :],
                                     xt_sb[:, 0, :], start=False, stop=False)
                    nc.tensor.matmul(g_ps[:, q, :], w_iht[:, 1, q, :],
                                     xt_sb[:, 1, :], start=False, stop=True)

                # gate activations: sigmoid(x) = 0.5 + 0.5 tanh(x/2)
                tg = small.tile([128, 4, BL], f32, tag="tg")
                for q in (0, 1, 3):
                    nc.scalar.activation(tg[:, q, :], g_ps[:, q, :], AF.Tanh,
                                         bias=hbias[:, q:q + 1], scale=0.5)
                nc.scalar.activation(tg[:, 2, :], g_ps[:, 2, :], AF.Tanh,
                                     bias=fbias[:, 2:3], scale=1.0)
                ug = small.tile([128, 3, BL], f32, tag="ug")  # u_i, u_f, u_o
                for qi, q in enumerate((0, 1, 3)):
                    nc.vector.tensor_scalar(out=ug[:, qi, :], in0=tg[:, q, :],
                                            scalar1=0.5, scalar2=0.5,
                                            op0=ALU.mult, op1=ALU.add)

                m1 = small.tile([128, BL], f32, tag="m1")
                nc.vector.tensor_tensor(out=m1[:], in0=ug[:, 1, :], in1=c_T[:],
                                        op=ALU.mult)
                m2 = small.tile([128, BL], f32, tag="m2")
                nc.vector.tensor_tensor(out=m2[:], in0=ug[:, 0, :],
                                        in1=tg[:, 2, :], op=ALU.mult)
                c_new = statep.tile([128, BL], f32, tag="cT")
                nc.vector.tensor_tensor(out=c_new[:], in0=m1[:], in1=m2[:],
                                        op=ALU.add)
                tc2 = small.tile([128, BL], f32, tag="tc2")
                nc.scalar.activation(tc2[:], c_new[:], AF.Tanh)
                h_new = statep.tile([128, BL], f32, tag="hT")
                nc.vector.tensor_tensor(out=h_new[:], in0=ug[:, 2, :],
                                        in1=tc2[:], op=ALU.mult)
                h_T, c_T = h_new, c_new

                # output staging: h2_bt = h_T.T -> stage, DMA every 8 steps
                hbt_ps = misc[0:BL, 192:320]
                nc.tensor.transpose(hbt_ps, h_T[:], ident[:])
                if t % 8 == 0:
                    stage = stagep.tile([BL, 8, 128], f32, tag="stage")
                nc.vector.tensor_copy(stage[:, t % 8, :], hbt_ps)
                if t % 8 == 7 or t == T_STEPS - 1:
                    t0 = (t // 8) * 8
                    nc.sync.dma_start(H_out[:, t0:t + 1, :],
                                      stage[:, :t + 1 - t0, :])

    nc.finalize()
    return nc


_NC_CACHE = {}


def _get_nc():
    if "nc" not in _NC_CACHE:
        _NC_CACHE["nc"] = _build()
    return _NC_CACHE["nc"]


def _prep_weights(W_e, v_e, W_ih, W_hh, b_ih, b_hh):
    W_hs, W_x = W_e[:, :2 * M], W_e[:, 2 * M:]
    W_hsT = np.ascontiguousarray(W_hs.T)             # [2m, s]
    w_hst = np.ascontiguousarray(
        W_hsT.reshape(2, 128, 128).transpose(1, 0, 2))  # [j, c, s]
    w_xt = np.ascontiguousarray(W_x.T)               # [t, s]
    W_ihT = np.ascontiguousarray(W_ih.T)             # [n=256, j=512]
    w_iht = np.ascontiguousarray(
        W_ihT.reshape(2, 128, 4, 128).transpose(1, 0, 2, 3))  # [n', h, q, j']
    W_hhT = np.ascontiguousarray(W_hh.T)             # [m, j=512]
    w_hht = np.ascontiguousarray(
        W_hhT.reshape(128, 4, 128))                  # [m, q, j']
    v = v_e[0].astype(np.float32)
    v_hi = v.astype(ml_dtypes.bfloat16)
    v_lo = (v - v_hi.astype(np.float32)).astype(ml_dtypes.bfloat16)
    v_pair = np.ascontiguousarray(np.stack([v_hi, v_lo], axis=1))
    bias = (b_ih + b_hh).astype(np.float32)
    hbias = np.ascontiguousarray(0.5 * bias.reshape(4, 128).T)
    fbias = np.ascontiguousarray(bias.reshape(4, 128).T)
    ident = np.eye(128, dtype=np.float32)
    ones_col = np.ones((128, 1), np.float32)
    ones_row = np.ones((1, 128), np.float32)
    pairmat = np.zeros((128, BL), np.float32)
    pairmat[np.arange(128), np.arange(128) // 2] = 1.0
    return dict(w_xt=w_xt, w_hst=w_hst, w_iht=w_iht, w_hht=w_hht,
                v_pair=v_pair, hbias=hbias, fbias=fbias, ident=ident,
                ones_col=ones_col, ones_row=ones_row, pairmat=pairmat)


def kernel(X, W_e, v_e, W_ih, W_hh, b_ih, b_hh, _trace=False, _tmpdir=None):
    X = np.ascontiguousarray(np.asarray(X, dtype=np.float32))
    wd = _prep_weights(np.asarray(W_e, np.float32), np.asarray(v_e, np.float32),
                       np.asarray(W_ih, np.float32), np.asarray(W_hh, np.float32),
                       np.asarray(b_ih, np.float32), np.asarray(b_hh, np.float32))
    nc = _get_nc()
    in_maps = []
    for core in range(NCORES):
        m = dict(wd)
        m["x"] = np.ascontiguousarray(X[core * BL:(core + 1) * BL])
        in_maps.append(m)
    kw = {}
    if _trace:
        kw = dict(trace=True, tmpdir=_tmpdir)
    res = run_bass_kernel_spmd(nc, in_maps, core_ids=list(range(NCORES)), **kw)
    out = np.concatenate(
        [res.results[c]["h_out"].transpose(1, 0, 2) for c in range(NCORES)],
        axis=1)
    if _trace:
        return out, res
    return out


# revision 12
# speedup vs baseline: 1.0753x; 1.0005x over previous
"""Trainium2 Bass kernel for the input-attention LSTM encoder (DA-RNN style).

Shapes (hardcoded): B=512, T=128, N=256, M=128. 8 NeuronCores, data-parallel
over batch (B_loc=64 per core), recurrent T-loop local per core.

Per core layout:
  P_sb  [s=128, b=64, n=256]  feat_proj, s on partitions (SBUF resident)
  XT_sb [n'=128, h=2, b=64, t=128]  X transposed (for x_t in [n, b] layout)
  state h_T, c_T [feat=128, b=64]  (feature-major so gate bias is per-partition)

Per step t:
  a   = W_hs_h @ h + W_hs_c @ c                  (PE)    [s, b]
  Z   = P + bcast(a)                             (DVE)   [s, b, n]
  Y   = tanh(Z) -> bf16                          (ACT)
  E_T[p, 2b+h_half] = sum_s v_s Y[s, b, 128h+p]  (PE, 128 small matmuls,
                                                  v split hi/lo bf16, N=2)
  expE = exp(E_hi) * exp(E_lo)                   (ACT + DVE)
  softmax denom via ones/pairmat matmuls + DVE reciprocal
  x_tilde = X[:, :, t] * expE * (1/S)            (DVE)
  gates = W_ih @ x_tilde + W_hh @ h  (+bias via ACT per-partition bias)
  sigmoid via 0.5 + 0.5*tanh(x/2)  -> LSTM update (DVE/ACT)
"""

import os
import numpy as np
import ml_dtypes

import concourse.bacc as bacc
import concourse.bass as bass
import concourse.mybir as mybir
import concourse.tile as tile
from concourse.bass_utils import run_bass_kernel_spmd

f32 = mybir.dt.float32
bf16 = mybir.dt.bfloat16
AF = mybir.ActivationFunctionType
ALU = mybir.AluOpType

B, T, N, M = 512, 128, 256, 128
NCORES = 8
BL = B // NCORES          # 64 batch per core
NCH = 8                   # chunks per step over b (8 b's each)
BCH = BL // NCH           # 16
T_STEPS = int(os.environ.get("K_STEPS", str(T)))


def _build(trace_friendly=False):
    nc = bacc.Bacc("TRN2", target_bir_lowering=False)

    X_in = nc.dram_tensor("x", [BL, T, N], f32, kind="ExternalInput")
    W_xt = nc.dram_tensor("w_xt", [128, 128], f32, kind="ExternalInput")
    W_hst = nc.dram_tensor("w_hst", [128, 2, 128], f32, kind="ExternalInput")
    W_iht = nc.dram_tensor("w_iht", [128, 2, 4, 128], f32, kind="ExternalInput")
    W_hht = nc.dram_tensor("w_hht", [128, 4, 128], f32, kind="ExternalInput")
    V_pair = nc.dram_tensor("v_pair", [128, 2], bf16, kind="ExternalInput")
    HBias = nc.dram_tensor("hbias", [128, 4], f32, kind="ExternalInput")
    FBias = nc.dram_tensor("fbias", [128, 4], f32, kind="ExternalInput")
    Ident = nc.dram_tensor("ident", [128, 128], f32, kind="ExternalInput")
    OnesC = nc.dram_tensor("ones_col", [128, 1], f32, kind="ExternalInput")
    OnesR = nc.dram_tensor("ones_row", [1, 128], f32, kind="ExternalInput")
    PairM = nc.dram_tensor("pairmat", [128, BL], f32, kind="ExternalInput")
    H_out = nc.dram_tensor("h_out", [BL, T, M], f32, kind="ExternalOutput")

    with tile.TileContext(nc) as tc:
        with tc.tile_pool(name="const", bufs=1) as cpool, \
             tc.tile_pool(name="big", bufs=1) as bigpool, \
             tc.tile_pool(name="work", bufs=4) as work, \
             tc.tile_pool(name="ybuf", bufs=4) as ybuf, \
             tc.tile_pool(name="small", bufs=4) as small, \
             tc.tile_pool(name="state", bufs=3) as statep, \
             tc.tile_pool(name="stage", bufs=2) as stagep, \
             tc.tile_pool(name="ps_e", bufs=2, space="PSUM") as psp_e, \
             tc.tile_pool(name="ps_g", bufs=2, space="PSUM") as psp_g, \
             tc.tile_pool(name="ps_a", bufs=2, space="PSUM") as psp_a, \
             tc.tile_pool(name="ps_m", bufs=2, space="PSUM") as psp_m:

            # ---- constants to SBUF ----
            w_xt = cpool.tile([128, 128], f32)
            w_hst = cpool.tile([128, 2, 128], f32)
            w_iht = cpool.tile([128, 2, 4, 128], f32)
            w_hht = cpool.tile([128, 4, 128], f32)
            v_pair = cpool.tile([128, 2], bf16)
            hbias = cpool.tile([128, 4], f32)
            fbias = cpool.tile([128, 4], f32)
            ident = cpool.tile([128, 128], f32)
            ones_col = cpool.tile([128, 1], f32)
            ones_row = cpool.tile([1, 128], f32)
            pairmat = cpool.tile([128, BL], f32)
            for dst, src in [(w_xt, W_xt), (w_hst, W_hst), (w_iht, W_iht),
                             (w_hht, W_hht), (v_pair, V_pair), (hbias, HBias),
                             (fbias, FBias), (ident, Ident), (ones_col, OnesC),
                             (ones_row, OnesR), (pairmat, PairM)]:
                nc.sync.dma_start(dst[:], src[:])

            P_sb = bigpool.tile([128, BL, N], bf16)      # [s, b, n] bf16
            XT_sb = bigpool.tile([128, 2, BL, T], f32)   # [n', h, b, t]

            # ---- preamble: load X, compute P = W_x @ X_b, transpose X ----
            for q in range(NCH):
                b0 = q * BCH
                xtn = work.tile([128, BCH, N], f32, tag="work")
                nc.sync.dma_start(
                    xtn[:], X_in[b0:b0 + BCH].rearrange("b t n -> t b n"))
                # P for 2 b's at a time (N=512 moving limit)
                for i in range(BCH // 2):
                    pp = psp_e.tile([128, 512], f32, tag="e")
                    nc.tensor.matmul(
                        pp[:], w_xt[:],
                        xtn[:, 2 * i:2 * i + 2, :].rearrange("p b n -> p (b n)"),
                        start=True, stop=True)
                    nc.scalar.copy(
                        P_sb[:, b0 + 2 * i:b0 + 2 * i + 2, :]
                        .rearrange("p b n -> p (b n)"), pp[:])
                # transpose X[b] -> XT, batches of 4 [128,128] blocks per copy
                for i in range(BCH // 2):
                    tp = psp_g.tile([128, 4, 128], f32, tag="g")
                    for j in range(2):      # b-offset within pair
                        for h in range(2):  # n half
                            nc.tensor.transpose(
                                tp[:, 2 * j + h, :],
                                xtn[:, 2 * i + j, 128 * h:128 * h + 128],
                                ident[:])
                    bb = b0 + 2 * i
                    nc.vector.tensor_copy(
                        XT_sb[:, :, bb:bb + 2, :].rearrange("p h b t -> p b h t"),
                        tp[:].rearrange("p (b h) t -> p b h t", b=2))

            # ---- state init ----
            h_T = statep.tile([128, BL], f32, tag="hT")
            c_T = statep.tile([128, BL], f32, tag="cT")
            nc.vector.memset(h_T[:], 0.0)
            nc.vector.memset(c_T[:], 0.0)

            # ---- recurrent steps ----
            for t_raw in range(T_STEPS):
                t = t_raw % T
                # a[s, b] = W_hs_h @ h + W_hs_c @ c
                a_ps = psp_a.tile([128, BL], f32, tag="a")
                nc.tensor.matmul(a_ps[:], w_hst[:, 0, :], h_T[:],
                                 start=True, stop=False)
                nc.tensor.matmul(a_ps[:], w_hst[:, 1, :], c_T[:],
                                 start=False, stop=True)
                a2 = small.tile([128, BL, 2], bf16, tag="a2")
                nc.scalar.copy(a2[:], a_ps[:].broadcast_to((128, BL, 2)))

                e_ps = psp_e.tile([128, 128, 2], f32, tag="e")
                for k in range(NCH):
                    bk = k * BCH
                    z = work.tile([128, BCH, N], bf16, tag="work")
                    a_bc = (a2[:, bk:bk + BCH, :]
                            .broadcast_to((128, BCH, 2, N // 2))
                            .rearrange("p b two r -> p b r two"))
                    nc.vector.tensor_tensor(
                        out=z[:].rearrange("p b (r two) -> p b r two", two=2),
                        in0=P_sb[:, bk:bk + BCH, :]
                        .rearrange("p b (r two) -> p b r two", two=2),
                        in1=a_bc, op=ALU.add)
                    y = ybuf.tile([128, BCH * N], bf16, tag="y")
                    nc.scalar.activation(y[:], z[:].rearrange("p b n -> p (b n)"),
                                         AF.Tanh)
                    for c in range(BCH * N // 128):  # col-blocks of 128
                        cc = k * (BCH * N // 128) + c
                        nc.tensor.matmul(e_ps[:, cc, :],
                                         y[:, 128 * c:128 * c + 128],
                                         v_pair[:], start=True, stop=True)

                # softmax pieces
                expp = small.tile([128, 128, 2], f32, tag="expp")
                nc.scalar.activation(expp[:], e_ps[:], AF.Exp)
                expE = small.tile([128, 128], f32, tag="expE")
                nc.vector.tensor_tensor(out=expE[:], in0=expp[:, :, 0],
                                        in1=expp[:, :, 1], op=ALU.mult)
                misc = psp_m.tile([128, 512], f32, tag="m")
                s2_ps = misc[:, 0:1]
                nc.tensor.matmul(s2_ps, expE[:], ones_col[:],
                                 start=True, stop=True)
                s2_sb = small.tile([128, 1], f32, tag="s2sb")
                nc.vector.tensor_copy(s2_sb[:], s2_ps)
                s_ps = misc[0:1, 64:64 + BL]
                nc.tensor.matmul(s_ps, s2_sb[:], pairmat[:],
                                 start=True, stop=True)
                r_sb = small.tile([1, BL], f32, tag="r")
                nc.vector.reciprocal(r_sb[:], s_ps)
                rrep_ps = misc[:, 128:128 + BL]
                nc.tensor.matmul(rrep_ps, ones_row[:], r_sb[:],
                                 start=True, stop=True)

                # x_tilde[h][n', b] = X[n, b, t] * expE[n', 2b+h] / S[b]
                u_sb = small.tile([128, 2, BL], f32, tag="u")
                nc.vector.tensor_tensor(
                    out=u_sb[:], in0=XT_sb[:, :, :, t],
                    in1=expE[:].rearrange("p (b h) -> p h b", h=2),
                    op=ALU.mult)
                xt_sb = small.tile([128, 2, BL], f32, tag="xt")
                nc.vector.tensor_tensor(
                    out=xt_sb[:], in0=u_sb[:],
                    in1=rrep_ps.broadcast_to((128, BL, 2))
                    .rearrange("p b h -> p h b"),
                    op=ALU.mult)

                # gates[j, b] = W_ih @ x_tilde + W_hh @ h
                g_ps = psp_g.tile([128, 4, BL], f32, tag="g")
                for q in range(4):
                    nc.tensor.matmul(g_ps[:, q, :], w_hht[:, q, :], h_T[:],
                                     start=True, stop=False)
                    nc.tensor.matmul(g_ps[:, q, :], w_iht[:, 0, q, :],
                                     xt_sb[:, 0, :], start=False, stop=False)
                    nc.tensor.matmul(g_ps[:, q, :], w_iht[:, 1, q, :],
                                     xt_sb[:, 1, :], start=False, stop=True)

                # gate activations: sigmoid(x) = 0.5 + 0.5 tanh(x/2)
                tg = small.tile([128, 4, BL], f32, tag="tg")
                for q in (0, 1, 3):
                    nc.scalar.activation(tg[:, q, :], g_ps[:, q, :], AF.Tanh,
                                         bias=hbias[:, q:q + 1], scale=0.5)
                nc.scalar.activation(tg[:, 2, :], g_ps[:, 2, :], AF.Tanh,
                                     bias=fbias[:, 2:3], scale=1.0)
                ug = small.tile([128, 3, BL], f32, tag="ug")  # u_i, u_f, u_o
                for qi, q in enumerate((0, 1, 3)):
                    nc.vector.tensor_scalar(out=ug[:, qi, :], in0=tg[:, q, :],
                                            scalar1=0.5, scalar2=0.5,
                                            op0=ALU.mult, op1=ALU.add)

                m1 = small.tile([128, BL], f32, tag="m1")
                nc.vector.tensor_tensor(out=m1[:], in0=ug[:, 1, :], in1=c_T[:],
                                        op=ALU.mult)
                m2 = small.tile([128, BL], f32, tag="m2")
                nc.vector.tensor_tensor(out=m2[:], in0=ug[:, 0, :],
                                        in1=tg[:, 2, :], op=ALU.mult)
                c_new = statep.tile([128, BL], f32, tag="cT")
                nc.vector.tensor_tensor(out=c_new[:], in0=m1[:], in1=m2[:],
                                        op=ALU.add)
                tc2 = small.tile([128, BL], f32, tag="tc2")
                nc.scalar.activation(tc2[:], c_new[:], AF.Tanh)
                h_new = statep.tile([128, BL], f32, tag="hT")
                nc.vector.tensor_tensor(out=h_new[:], in0=ug[:, 2, :],
                                        in1=tc2[:], op=ALU.mult)
                h_T, c_T = h_new, c_new

                # output staging: h2_bt = h_T.T -> stage, DMA every 8 steps
                hbt_ps = misc[0:BL, 192:320]
                nc.tensor.transpose(hbt_ps, h_T[:], ident[:])
                if t % 8 == 0:
                    stage = stagep.tile([BL, 8, 128], f32, tag="stage")
                nc.vector.tensor_copy(stage[:, t % 8, :], hbt_ps)
                if t % 8 == 7 or t == T_STEPS - 1:
                    t0 = (t // 8) * 8
                    nc.sync.dma_start(H_out[:, t0:t + 1, :],
                                      stage[:, :t + 1 - t0, :])

    nc.finalize()
    return nc


_NC_CACHE = {}


def _get_nc():
    if "nc" not in _NC_CACHE:
        _NC_CACHE["nc"] = _build()
    return _NC_CACHE["nc"]


def _prep_weights(W_e, v_e, W_ih, W_hh, b_ih, b_hh):
    W_hs, W_x = W_e[:, :2 * M], W_e[:, 2 * M:]
    W_hsT = np.ascontiguousarray(W_hs.T)             # [2m, s]
    w_hst = np.ascontiguousarray(
        W_hsT.reshape(2, 128, 128).transpose(1, 0, 2))  # [j, c, s]
    w_xt = np.ascontiguousarray(W_x.T)               # [t, s]
    W_ihT = np.ascontiguousarray(W_ih.T)             # [n=256, j=512]
    w_iht = np.ascontiguousarray(
        W_ihT.reshape(2, 128, 4, 128).transpose(1, 0, 2, 3))  # [n', h, q, j']
    W_hhT = np.ascontiguousarray(W_hh.T)             # [m, j=512]
    w_hht = np.ascontiguousarray(
        W_hhT.reshape(128, 4, 128))                  # [m, q, j']
    v = v_e[0].astype(np.float32)
    v_hi = v.astype(ml_dtypes.bfloat16)
    v_lo = (v - v_hi.astype(np.float32)).astype(ml_dtypes.bfloat16)
    v_pair = np.ascontiguousarray(np.stack([v_hi, v_lo], axis=1))
    bias = (b_ih + b_hh).astype(np.float32)
    hbias = np.ascontiguousarray(0.5 * bias.reshape(4, 128).T)
    fbias = np.ascontiguousarray(bias.reshape(4, 128).T)
    ident = np.eye(128, dtype=np.float32)
    ones_col = np.ones((128, 1), np.float32)
    ones_row = np.ones((1, 128), np.float32)
    pairmat = np.zeros((128, BL), np.float32)
    pairmat[np.arange(128), np.arange(128) // 2] = 1.0
    return dict(w_xt=w_xt, w_hst=w_hst, w_iht=w_iht, w_hht=w_hht,
                v_pair=v_pair, hbias=hbias, fbias=fbias, ident=ident,
                ones_col=ones_col, ones_row=ones_row, pairmat=pairmat)


def kernel(X, W_e, v_e, W_ih, W_hh, b_ih, b_hh, _trace=False, _tmpdir=None):
    X = np.ascontiguousarray(np.asarray(X, dtype=np.float32))
    wd = _prep_weights(np.asarray(W_e, np.float32), np.asarray(v_e, np.float32),
                       np.asarray(W_ih, np.float32), np.asarray(W_hh, np.float32),
                       np.asarray(b_ih, np.float32), np.asarray(b_hh, np.float32))
    nc = _get_nc()
    in_maps = []
    for core in range(NCORES):
        m = dict(wd)
        m["x"] = np.ascontiguousarray(X[core * BL:(core + 1) * BL])
        in_maps.append(m)
    kw = {}
    if _trace:
        kw = dict(trace=True, tmpdir=_tmpdir)
    res = run_bass_kernel_spmd(nc, in_maps, core_ids=list(range(NCORES)), **kw)
    out = np.concatenate(
        [res.results[c]["h_out"].transpose(1, 0, 2) for c in range(NCORES)],
        axis=1)
    if _trace:
        return out, res
    return out
